# revision 19
# baseline (speedup 1.0000x reference)
"""Cross multi-head attention kernel for 8 Trainium2 NeuronCores (v2).

Reference computation (per batch b):
    Q = x @ Wq.T ; K = ctx @ Wk.T ; V = ctx @ Wv.T          (16 heads, depth 64)
    scores = (Q_h @ K_h.T) / 8 ; masked where pad_mask -> -inf
    att = softmax(scores) ; out_h = att @ V_h
    y = concat_h(out_h) @ fc_w.T + fc_b

Sharding: 8 cores = 2 batches x 4 head-groups (4 heads each).  Each core
computes a full [E, LQ] partial of y^T for its batch; the host sums the 4
head-group partials per batch and adds the bias.

v2 changes vs the 206us baseline (trace-driven):
  * all inputs bf16 (halves DMA; f32r matmuls -> bf16 matmuls).
  * fully software-pipelined single instruction stream: the V projection and
    the second pair's Q/K projections are interleaved into the attention kt
    loop so the PE never idles (idle PE drops to the 0.65/1.2 GHz p-states;
    continuously busy reaches 2.4 GHz).
  * softmax normalization without Ln/Exp ACT table swaps or PE broadcast
    matmuls: rowsum reciprocal via 2 Newton steps on DVE seeded at the
    analytic center (rowsums concentrate near keep*LKV*E[exp N(0,1)]), then
    gpsimd partition_broadcast, then one fused (av * -1) * (-recip) DVE op.
  * AV matmuls issued with a one-kt skew after the next tile's scores so the
    exp/mask latency is hidden behind PE work.
  * engine balance: exp on ACT; mask mult h0 on DVE, h1 on gpsimd; V/proj
    evacuations on ACT; fc evacuations round-robin over DVE/gpsimd/ACT.
"""

import os
import sys

import numpy as np

for _p in ("/opt/trn_rl_repo", "/root/.axon_site/_ro/trn_rl_repo"):
    if os.path.isdir(_p) and _p not in sys.path:
        sys.path.insert(0, _p)

import ml_dtypes  # noqa: E402

import concourse.bass as bass  # noqa: E402
import concourse.mybir as mybir  # noqa: E402
import concourse.tile as tile  # noqa: E402
from concourse import bacc  # noqa: E402
from concourse.bass_utils import run_bass_kernel_spmd  # noqa: E402

B, LQ, LKV, E = 2, 1024, 2048, 1024
H_TOTAL, D = 16, 64
NCORES = 8
HGROUPS = 4          # head groups (cores per batch)
HLOCAL = 4           # heads per core
FP = HLOCAL * D      # 256 local head features
P = 128
F32 = mybir.dt.float32
BF16 = mybir.dt.bfloat16
ET = E // P          # 8 contraction tiles for the projections
KT = LKV // P        # 16 key tiles
NQ = LQ // 512       # 2 matmul free-dim chunks


def build_nc(debug: bool = False) -> bass.Bass:
    nc = bacc.Bacc("TRN2", target_bir_lowering=False)

    xT = nc.dram_tensor("xT", [E, LQ], BF16, kind="ExternalInput")
    ctxT = nc.dram_tensor("ctxT", [E, LKV], BF16, kind="ExternalInput")
    maskT = nc.dram_tensor("maskT", [LKV, LQ], BF16, kind="ExternalInput")
    wqT = nc.dram_tensor("wqT", [E, FP], BF16, kind="ExternalInput")
    wkT = nc.dram_tensor("wkT", [E, FP], BF16, kind="ExternalInput")
    wvT = nc.dram_tensor("wvT", [E, FP], BF16, kind="ExternalInput")
    fcwT = nc.dram_tensor("fcwT", [FP, E], BF16, kind="ExternalInput")
    yT = nc.dram_tensor("yT", [E, LQ], BF16, kind="ExternalOutput")
    if debug:
        qt_dbg = nc.dram_tensor("qt_dbg", [P, 2, LQ], BF16, kind="ExternalOutput")
        kt_dbg = nc.dram_tensor("kt_dbg", [P, 2, LKV], BF16, kind="ExternalOutput")
        va_dbg = nc.dram_tensor("va_dbg", [P, KT, HLOCAL, D + 1], BF16, kind="ExternalOutput")
        ex_dbg = nc.dram_tensor("ex_dbg", [2, P, LQ], BF16, kind="ExternalOutput")
        avs_dbg = nc.dram_tensor("avs_dbg", [P, LQ], F32, kind="ExternalOutput")
        rsw_dbg = nc.dram_tensor("rsw_dbg", [2, LQ], F32, kind="ExternalOutput")
        rcw_dbg = nc.dram_tensor("rcw_dbg", [2, LQ], F32, kind="ExternalOutput")
        rbc_dbg = nc.dram_tensor("rbc_dbg", [P, LQ], F32, kind="ExternalOutput")
        at_dbg = nc.dram_tensor("at_dbg", [P, 2, LQ], BF16, kind="ExternalOutput")

    mult = mybir.AluOpType.mult
    add = mybir.AluOpType.add

    with tile.TileContext(nc) as tc:
        with (
            tc.tile_pool(name="persist", bufs=1) as persist,
            tc.tile_pool(name="inp", bufs=1) as inp,
            tc.tile_pool(name="maskp", bufs=1) as maskp,
            tc.tile_pool(name="ex", bufs=4) as exp_pool,
            tc.tile_pool(name="outp", bufs=3) as outp,
        ):
            QT = persist.tile([P, 2, LQ], BF16)        # [:, pair, :]; head 2p on rows 0:64
            KTt = persist.tile([P, 2, LKV], BF16)
            Vaug = persist.tile([P, KT, HLOCAL, D + 1], BF16)
            attnT = persist.tile([P, 2, LQ], BF16)
            fcw_s = persist.tile([P, 2, E], BF16)
            # partition_broadcast ucode honours neither input nor output
            # partition offsets, so every broadcast source/target (and the
            # tensor_tensor inputs it feeds) lives in its own partition-0 tile.
            av_sp = [persist.tile([D, LQ], F32, name=f"av_sp{i}") for i in range(2)]
            rsw = persist.tile([P, LQ], F32)   # rowsums packed at partitions 0 / 64
            rcw = persist.tile([P, LQ], F32)   # their reciprocals
            rbcp = [persist.tile([D, LQ], F32, name=f"rbcp{i}") for i in range(2)]
            rc1 = persist.tile([1, LQ], F32)   # partition-0 copy of the h1 recip row

            nc.gpsimd.memset(Vaug[:], 1.0)            # ones column survives; V overwrites the rest

            # ---------------- input DMAs, in consumption order ----------------
            wq_s, wk_s, wv_s, xT_s, cT_s, mT_s = [], [], [], [], [], []
            for k in range(ET):
                wq = inp.tile([P, FP], BF16, tag=f"wq{k}")
                nc.sync.dma_start(wq[:], wqT[k * P:(k + 1) * P, :])
                wq_s.append(wq)
            for k in range(ET):
                xt = inp.tile([P, LQ], BF16, tag=f"xT{k}")
                xT_s.append(xt)
            for n in range(NQ):
                for k in range(ET):
                    nc.sync.dma_start(
                        xT_s[k][:, n * 512:(n + 1) * 512],
                        xT[k * P:(k + 1) * P, n * 512:(n + 1) * 512],
                    )
            for k in range(ET):
                wk = inp.tile([P, FP], BF16, tag=f"wk{k}")
                nc.sync.dma_start(wk[:], wkT[k * P:(k + 1) * P, :])
                wk_s.append(wk)
                wv = inp.tile([P, FP], BF16, tag=f"wv{k}")
                nc.sync.dma_start(wv[:], wvT[k * P:(k + 1) * P, :])
                wv_s.append(wv)
            for k in range(ET):
                ct = inp.tile([P, LKV], BF16, tag=f"cT{k}")
                cT_s.append(ct)
            for n in range(LKV // 512):
                for k in range(ET):
                    nc.sync.dma_start(
                        cT_s[k][:, n * 512:(n + 1) * 512],
                        ctxT[k * P:(k + 1) * P, n * 512:(n + 1) * 512],
                    )
            nc.sync.dma_start(fcw_s[:], fcwT.rearrange("(ko pi) e -> pi ko e", pi=P))
            for kt in range(KT):
                mt = maskp.tile([P, LQ], BF16, tag=f"m{kt}")
                nc.sync.dma_start(mt[:], maskT[kt * P:(kt + 1) * P, :])
                mT_s.append(mt)

            # ---------------- helper emitters ----------------
            def proj_qk(dst, dst_p, w_s, src_s, n, width):
                """One [128, width] column chunk of Q^T/K^T pair dst_p."""
                ps = psum.tile([P, 512], F32, tag="sc", bufs=3)
                for k in range(ET):
                    nc.tensor.matmul(
                        ps[:, 0:width],
                        w_s[k][:, dst_p * P:(dst_p + 1) * P],
                        src_s[k][:, n * width:(n + 1) * width],
                        start=(k == 0),
                        stop=(k == ET - 1),
                    )
                nc.scalar.copy(dst[:, dst_p, n * width:(n + 1) * width], ps[:, 0:width])

            def proj_v(mv):
                """V rows [mv*128, (mv+1)*128) natural, all 4 heads, into Vaug."""
                ps = psum.tile([P, 512], F32, tag="proj", bufs=1)
                for k in range(ET):
                    nc.tensor.matmul(
                        ps[:, 0:FP],
                        cT_s[k][:, mv * P:(mv + 1) * P],
                        wv_s[k][:],
                        start=(k == 0),
                        stop=(k == ET - 1),
                    )
                nc.scalar.copy(
                    Vaug[:, mv, :, 0:D],
                    ps[:, 0:FP].rearrange("p (h d) -> p h d", d=D),
                )

            def proj_qk_jit(dst, dst_p, w_s, src_s, n, width):
                """Late Q/K chunks through the 1-buf proj psum tag."""
                ps = psum.tile([P, 512], F32, tag="proj", bufs=1)
                for k in range(ET):
                    nc.tensor.matmul(
                        ps[:, 0:width],
                        w_s[k][:, dst_p * P:(dst_p + 1) * P],
                        src_s[k][:, n * width:(n + 1) * width],
                        start=(k == 0),
                        stop=(k == ET - 1),
                    )
                nc.scalar.copy(dst[:, dst_p, n * width:(n + 1) * width], ps[:, 0:width])

            def scores_exp_mask(p, kt, h):
                """scores -> exp -> mask for (pair p, key tile kt, head h).
                Returns the masked-exp tile [128, LQ] bf16."""
                base = h * D
                ex = exp_pool.tile([P, LQ], BF16, tag="ex")
                for n in range(NQ):
                    sc = psum.tile([P, 512], F32, tag="sc", bufs=3)
                    nc.tensor.matmul(
                        sc[:],
                        KTt[base:base + D, p, kt * P:(kt + 1) * P],
                        QT[base:base + D, p, n * 512:(n + 1) * 512],
                        start=True,
                        stop=True,
                    )
                    nc.scalar.activation(
                        ex[:, n * 512:(n + 1) * 512], sc[:],
                        mybir.ActivationFunctionType.Exp,
                        scale=0.125,
                    )
                nc.vector.tensor_tensor(ex[:], ex[:], mT_s[kt][:], mult)
                return ex

            def av_accum(avs, p, kt, h, ex):
                for n in range(NQ):
                    nc.tensor.matmul(
                        avs[h][:, n * 512:(n + 1) * 512],
                        Vaug[:, kt, 2 * p + h, :],
                        ex[:, n * 512:(n + 1) * 512],
                        start=(kt == 0),
                        stop=(kt == KT - 1),
                    )


            # ---------------- attention: scoped PSUM (sc 3 + proj 1 + av 2x2 = 8 banks) ----
            with tc.tile_pool(name="psumB", bufs=1, space="PSUM") as psum:
                # pair 0: upfront Q/K, then the kt pipeline
                for n in range(NQ):
                    proj_qk(QT, 0, wq_s, xT_s, n, 512)
                for n in range(LKV // 512):
                    proj_qk(KTt, 0, wk_s, cT_s, n, 512)

                avs = [
                    psum.tile([D + 1, LQ], F32, tag="av0", name="av0"),
                    psum.tile([D + 1, LQ], F32, tag="av1", name="av1"),
                ]
                pend = None
                for kt in range(KT):
                    proj_v(kt)
                    if kt == 13:
                        proj_qk_jit(QT, 1, wq_s, xT_s, 0, 512)
                    elif kt == 14:
                        proj_qk_jit(QT, 1, wq_s, xT_s, 1, 512)
                    elif kt == 15:
                        proj_qk_jit(KTt, 1, wk_s, cT_s, 0, 512)
                    exs = [scores_exp_mask(0, kt, h) for h in range(2)]
                    if debug and kt == 0:
                        for h in range(2):
                            nc.sync.dma_start(ex_dbg[h], exs[h][:])
                    if pend is not None:
                        for h in range(2):
                            av_accum(avs, 0, pend[0], h, pend[1][h])
                    pend = (kt, exs)
                for h in range(2):
                    av_accum(avs, 0, pend[0], h, pend[1][h])

                # evacuate pair-0 AV + rowsums so the psum banks free quickly
                nc.vector.tensor_copy(rsw[0:1, :], avs[0][D:D + 1, :])
                nc.vector.tensor_copy(rsw[D:D + 1, :], avs[1][D:D + 1, :])
                for h in range(2):
                    nc.vector.tensor_copy(av_sp[h][:], avs[h][0:D, :])

                # pair-0 normalization (fully overlapped with the pair-1 loop)
                nc.vector.reciprocal(rcw[0:D + 1, :], rsw[0:D + 1, :])
                nc.vector.tensor_copy(rc1[:], rcw[D:D + 1, :])
                nc.gpsimd.partition_broadcast(rbcp[0][:], rcw[0:1, :])
                nc.gpsimd.partition_broadcast(rbcp[1][:], rc1[:])
                for h in range(2):
                    nc.vector.tensor_tensor(
                        attnT[h * D:(h + 1) * D, 0, :],
                        av_sp[h][:], rbcp[h][:], mult,
                    )
                if debug:
                    nc.sync.dma_start(qt_dbg[:], QT[:])
                    nc.sync.dma_start(kt_dbg[:], KTt[:])
                    nc.sync.dma_start(va_dbg[:], Vaug[:])
                    nc.sync.dma_start(avs_dbg[0:D, :], av_sp[0][:])
                    nc.sync.dma_start(avs_dbg[D:2 * D, :], av_sp[1][:])
                    nc.sync.dma_start(rsw_dbg[0:1, :], rsw[0:1, :])
                    nc.sync.dma_start(rsw_dbg[1:2, :], rsw[D:D + 1, :])
                    nc.sync.dma_start(rcw_dbg[0:1, :], rcw[0:1, :])
                    nc.sync.dma_start(rcw_dbg[1:2, :], rcw[D:D + 1, :])
                    nc.sync.dma_start(rbc_dbg[0:D, :], rbcp[0][:])
                    nc.sync.dma_start(rbc_dbg[D:2 * D, :], rbcp[1][:])

                # pair 1 kt pipeline (K chunks just-in-time)
                avs1 = [
                    psum.tile([D + 1, LQ], F32, tag="av0", name="av0b"),
                    psum.tile([D + 1, LQ], F32, tag="av1", name="av1b"),
                ]
                pend = None
                for kt in range(KT):
                    if kt == 0:
                        proj_qk_jit(KTt, 1, wk_s, cT_s, 1, 512)
                    elif kt == 4:
                        proj_qk_jit(KTt, 1, wk_s, cT_s, 2, 512)
                    elif kt == 8:
                        proj_qk_jit(KTt, 1, wk_s, cT_s, 3, 512)
                    exs = [scores_exp_mask(1, kt, h) for h in range(2)]
                    if pend is not None:
                        for h in range(2):
                            av_accum(avs1, 1, pend[0], h, pend[1][h])
                    pend = (kt, exs)
                for h in range(2):
                    av_accum(avs1, 1, pend[0], h, pend[1][h])

                # pair-1 normalization straight out of psum; both broadcast
                # targets start at partition 0 to match avs1[h][0:D]
                nc.vector.tensor_copy(rsw[0:1, :], avs1[0][D:D + 1, :])
                nc.vector.tensor_copy(rsw[D:D + 1, :], avs1[1][D:D + 1, :])
                nc.vector.reciprocal(rcw[0:D + 1, :], rsw[0:D + 1, :])
                nc.vector.tensor_copy(rc1[:], rcw[D:D + 1, :])
                nc.gpsimd.partition_broadcast(rbcp[0][:], rcw[0:1, :])
                nc.gpsimd.partition_broadcast(rbcp[1][:], rc1[:])
                for h in range(2):
                    nc.vector.tensor_tensor(
                        attnT[h * D:(h + 1) * D, 1, :],
                        avs1[h][0:D, :], rbcp[h][:], mult,
                    )

            if debug:
                nc.sync.dma_start(at_dbg[:], attnT[:])

            # ---------------- output projection ----------------
            with tc.tile_pool(name="psumC", bufs=2, space="PSUM") as psumC:
                for m in range(ET):
                    ps = psumC.tile([P, LQ], F32, tag="fc")
                    for n in range(NQ):
                        for kf in range(2):
                            nc.tensor.matmul(
                                ps[:, n * 512:(n + 1) * 512],
                                fcw_s[:, kf, m * P:(m + 1) * P],
                                attnT[:, kf, n * 512:(n + 1) * 512],
                                start=(kf == 0),
                                stop=(kf == 1),
                            )
                    ob = outp.tile([P, LQ], BF16, tag="ob")
                    if m % 2 == 0:
                        nc.vector.tensor_copy(ob[:], ps[:])
                    else:
                        nc.scalar.copy(ob[:], ps[:])
                    nc.sync.dma_start(yT[m * P:(m + 1) * P, :], ob[:])

    nc.compile()
    return nc


_NC_CACHE: dict = {}


def _get_nc() -> bass.Bass:
    if "nc" not in _NC_CACHE:
        _NC_CACHE["nc"] = build_nc()
    return _NC_CACHE["nc"]


def make_in_maps(x, context, pad_mask, Wq, Wk, Wv, fc_w):
    BF = ml_dtypes.bfloat16
    x = np.asarray(x, dtype=np.float32)
    context = np.asarray(context, dtype=np.float32)
    pad_mask = np.asarray(pad_mask).astype(bool)
    Wq = np.asarray(Wq, dtype=np.float32)
    Wk = np.asarray(Wk, dtype=np.float32)
    Wv = np.asarray(Wv, dtype=np.float32)
    fc_w = np.asarray(fc_w, dtype=np.float32)

    xT = np.ascontiguousarray(x.transpose(0, 2, 1)).astype(BF)      # [B, E, LQ]
    cT = np.ascontiguousarray(context.transpose(0, 2, 1)).astype(BF)  # [B, E, LKV]
    keepT = np.ascontiguousarray(
        (~pad_mask).transpose(0, 2, 1)
    ).astype(BF)                                                    # [B, LKV, LQ]

    in_maps = []
    for c in range(NCORES):
        b, hg = divmod(c, HGROUPS)
        fsl = slice(hg * FP, (hg + 1) * FP)
        in_maps.append(
            {
                "xT": xT[b],
                "ctxT": cT[b],
                "maskT": keepT[b],
                "wqT": np.ascontiguousarray(Wq[fsl, :].T).astype(BF),
                "wkT": np.ascontiguousarray(Wk[fsl, :].T).astype(BF),
                "wvT": np.ascontiguousarray(Wv[fsl, :].T).astype(BF),
                "fcwT": np.ascontiguousarray(fc_w[:, fsl].T).astype(BF),
            }
        )
    return in_maps


def _combine(outs, fc_b):
    fc_b = np.asarray(fc_b, dtype=np.float32)
    y = np.empty((B, LQ, E), dtype=np.float32)
    for b in range(B):
        acc = outs[HGROUPS * b].astype(np.float32)
        for g in range(1, HGROUPS):
            acc = acc + outs[HGROUPS * b + g].astype(np.float32)
        y[b] = acc.T + fc_b
    return y


def run_traced(x, context, pad_mask, Wq, Wk, Wv, fc_w, fc_b, trace=False):
    nc = _get_nc()
    in_maps = make_in_maps(x, context, pad_mask, Wq, Wk, Wv, fc_w)
    res = run_bass_kernel_spmd(nc, in_maps, list(range(NCORES)), trace=trace)
    outs = [r["yT"] for r in res.results]
    return _combine(outs, fc_b), res


def kernel(x, context, pad_mask, Wq, Wk, Wv, fc_w, fc_b):
    y, _ = run_traced(x, context, pad_mask, Wq, Wk, Wv, fc_w, fc_b, trace=False)
    return y
